# revision 1
# baseline (speedup 1.0000x reference)
"""Trainium2 Bass kernel for nn_GetNodeK (gnn_message_passing).

out[b,i,n,m,:] = node_embedding[b, nbr_idx[b, nbr_idx[b,i,n], m], :]

Sharding: data-parallel over B (8 batches -> 8 cores, one batch per core).

Let nbr_flat = nbr_idx[b].reshape(6144) (values < 256) and define the
one-hop table G[j] = concat_m emb[nbr[j,m]] (256 rows x 12 KB = 3.1 MB).
Then out[b, k=(i*24+n)] = G[nbr_flat[k]] -- the 2-hop gather factors into
two index-driven stages that both use the raw nbr values (no chained
index arithmetic anywhere).

v2 (default): stage 1 dma_gather emb->G in SBUF (permuted so scatter-token
j sits at partition j%128, half j//128, 12 KB contiguous); stage 2 is
T = max_j count(j) rounds of indirect_dma_start scatter SBUF->DRAM where
round r writes G[j] to the r-th output row that references j (OOB-skip
via bounds_check for exhausted tokens). HBM traffic: 75.5 MB write +
3.1 MB read per core (roofline-ish).

v1 (fallback): stage 1 gather -> G -> DRAM; stage 2 dma_gather 12 KB rows
from G_dram -> SBUF tiles -> sequential DMA out. Extra 75.5 MB read.
"""
import numpy as np

from concourse import bass, bacc, mybir
import concourse.tile as tile
from concourse.bass_utils import run_bass_kernel_spmd

B, At, Nbr, F = 8, 256, 24, 128
NI = At * Nbr        # 6144 indices per batch
ROW = Nbr * F        # 3072 f32 = 12 KB per stage-2 row
CH = 512             # v1 stage-2 chunk (indices per gather call)
NCHUNK = NI // CH    # 12
OOB = 8192           # idx sentinel > NI-1 -> skipped by bounds_check

VERSION = "v2"
_CACHED = {}


# ---------------------------------------------------------------- v1 ----
def _build_nc_v1():
    nc = bacc.Bacc("TRN2", target_bir_lowering=False, debug=False)
    emb = nc.dram_tensor("emb", [At, F], mybir.dt.float32, kind="ExternalInput")
    gidx = nc.dram_tensor("gidx", [128, NI // 16], mybir.dt.int16, kind="ExternalInput")
    g_dram = nc.dram_tensor("g_scratch", [NI, F], mybir.dt.float32)
    out = nc.dram_tensor("out", [NI, ROW], mybir.dt.float32, kind="ExternalOutput")

    with tile.TileContext(nc) as tc:
        with tc.tile_pool(name="pool0", bufs=1) as pool0, \
             tc.tile_pool(name="pool2", bufs=2) as pool2:
            idx_t = pool0.tile([128, NI // 16], mybir.dt.int16)
            nc.sync.dma_start(idx_t[:], gidx[:])

            g_t = pool0.tile([128, NI // 128, F], mybir.dt.float32)
            nc.gpsimd.dma_gather(g_t[:], emb[:], idx_t[:], NI, NI, F,
                                 single_packet=False)
            nc.sync.dma_start(
                g_dram[:].rearrange("(s p) e -> p s e", p=128), g_t[:]
            )

            g_view = g_dram[:].rearrange("(j k) e -> j (k e)", k=Nbr)  # [256, 3072]
            for c in range(NCHUNK):
                t2 = pool2.tile([128, CH // 128, ROW], mybir.dt.float32, tag="t2")
                nc.gpsimd.dma_gather(
                    t2[:], g_view,
                    idx_t[:, c * (CH // 16):(c + 1) * (CH // 16)],
                    CH, CH, ROW,
                )
                nc.sync.dma_start(
                    out[c * CH:(c + 1) * CH].rearrange("(s p) e -> p s e", p=128),
                    t2[:],
                )
    nc.compile()
    return nc


def _prep_v1(nbr16_b):
    flat = nbr16_b.reshape(-1)
    return {"gidx": np.tile(flat.reshape(NI // 16, 16).T, (8, 1))}


# ---------------------------------------------------------------- v2 ----
_T_PERM = None


def _v1_perm():
    """idx1[t] = nbr[(t//128//24)*128 + t%128, (t//128)%24] as flat index."""
    global _T_PERM
    if _T_PERM is None:
        t = np.arange(NI)
        s, p = t // 128, t % 128
        j, m = (s // Nbr) * 128 + p, s % Nbr
        _T_PERM = j * Nbr + m
    return _T_PERM


def _prep_v2(nbr16_b, T):
    flat = nbr16_b.reshape(-1)
    idx1 = flat[_v1_perm()]
    gidx = np.tile(idx1.reshape(NI // 16, 16).T, (8, 1))

    counts = np.bincount(flat, minlength=At)
    order = np.argsort(flat, kind="stable")
    tbl = np.full((At, T), OOB, dtype=np.int32)
    pos = 0
    for j in range(At):
        c = counts[j]
        tbl[j, :c] = order[pos:pos + c]
        pos += c
    sidx = np.empty((128, T, 2), dtype=np.int32)
    for q in range(2):
        sidx[:, :, q] = tbl[q * 128:(q + 1) * 128, :]
    return {"gidx": gidx, "sidx": sidx}


def _build_nc_v2(T):
    nc = bacc.Bacc("TRN2", target_bir_lowering=False, debug=False)
    emb = nc.dram_tensor("emb", [At, F], mybir.dt.float32, kind="ExternalInput")
    gidx = nc.dram_tensor("gidx", [128, NI // 16], mybir.dt.int16, kind="ExternalInput")
    sidx = nc.dram_tensor("sidx", [128, T, 2], mybir.dt.int32, kind="ExternalInput")
    out = nc.dram_tensor("out", [NI, ROW], mybir.dt.float32, kind="ExternalOutput")

    with tile.TileContext(nc) as tc:
        with tc.tile_pool(name="pool0", bufs=1) as pool0:
            idx_t = pool0.tile([128, NI // 16], mybir.dt.int16)
            nc.sync.dma_start(idx_t[:], gidx[:])
            sidx_t = pool0.tile([128, T, 2], mybir.dt.int32)
            nc.sync.dma_start(sidx_t[:], sidx[:])

            g_t = pool0.tile([128, NI // 128, F], mybir.dt.float32)
            nc.gpsimd.dma_gather(g_t[:], emb[:], idx_t[:], NI, NI, F,
                                 single_packet=False)

            g_scatter = g_t[:].rearrange("p (q m) e -> p q (m e)", q=2)
            for r in range(T):
                for q in range(2):
                    nc.gpsimd.indirect_dma_start(
                        out=out[:],
                        out_offset=bass.IndirectOffsetOnAxis(
                            ap=sidx_t[:, r, q:q + 1], axis=0),
                        in_=g_scatter[:, q, :],
                        in_offset=None,
                        bounds_check=NI - 1,
                        oob_is_err=False,
                    )
    nc.compile()
    return nc


# ------------------------------------------------------------- driver ----
def _run(nc, in_maps, **kwargs):
    return run_bass_kernel_spmd(nc, in_maps, core_ids=list(range(B)), **kwargs)


def kernel(node_embedding: np.ndarray, nbr_idx: np.ndarray, _collect=None) -> np.ndarray:
    node_embedding = np.ascontiguousarray(node_embedding, dtype=np.float32)
    nbr16 = nbr_idx.astype(np.int16)  # values in [0, 256)

    if VERSION == "v1":
        if "v1" not in _CACHED:
            _CACHED["v1"] = _build_nc_v1()
        nc = _CACHED["v1"]
        in_maps = [{"emb": node_embedding[b], **_prep_v1(nbr16[b])}
                   for b in range(B)]
    else:
        T = int(max(np.bincount(nbr16[b].reshape(-1), minlength=At).max()
                    for b in range(B)))
        key = ("v2", T)
        if key not in _CACHED:
            _CACHED[key] = _build_nc_v2(T)
        nc = _CACHED[key]
        in_maps = [{"emb": node_embedding[b], **_prep_v2(nbr16[b], T)}
                   for b in range(B)]

    res = _run(nc, in_maps)
    if _collect is not None:
        _collect.append(res)
    outs = [res.results[b]["out"].reshape(At, Nbr, Nbr, F) for b in range(B)]
    return np.stack(outs, axis=0)



# revision 5
# speedup vs baseline: 1.7993x; 1.7993x over previous
"""Trainium2 Bass kernel for nn_GetNodeK (gnn_message_passing).

out[b,i,n,m,:] = node_embedding[b, nbr_idx[b, nbr_idx[b,i,n], m], :]

Sharding: data-parallel over B (8 batches -> 8 cores, one batch per core).

Let nbr_flat = nbr_idx[b].reshape(6144) (values < 256) and define the
one-hop table G[j] = concat_m emb[nbr[j,m]] (256 rows x 12 KB = 3.1 MB).
Then out[b, k=(i*24+n)] = G[nbr_flat[k]] -- the 2-hop gather factors into
two index-driven stages that both use the raw nbr values (no chained
index arithmetic anywhere).

v3 (default): stage 1 dma_gather emb->G in SBUF (permuted so scatter-token
j sits at partition j%128, half j//128, 12 KB contiguous), split into two
half gathers (tokens 0-127, 128-255). Stage 2 is ONE indirect_dma_start
per half: offset AP [128, T] (T = max occurrence count; row r of
partition p = r-th output row referencing token q*128+p, OOB-padded) and
a stride-0 broadcast input AP [128, T, 3072] so every descriptor of
partition p reads the same 12 KB row G[q*128+p]. Skipped (OOB) entries
cost only Q7 descriptor-gen time (~1 ns each); real descriptors drain on
all 16 SDMA engines back-to-back instead of the v2 scheme's ~90
serialized 128-descriptor rounds. HBM traffic: 75.5 MB write + 3.1 MB
read per core.

v2 (fallback): T rounds x 2 halves of indirect_dma_start scatter with
[128, 1] offsets -- serialized rounds, ~2.4x slower.
"""
import numpy as np

from concourse import bass, bacc, mybir
import concourse.tile as tile
from concourse.bass_utils import run_bass_kernel_spmd

B, At, Nbr, F = 8, 256, 24, 128
NI = At * Nbr        # 6144 indices per batch
ROW = Nbr * F        # 3072 f32 = 12 KB per stage-2 row
HALF = NI // 2       # 3072 gather indices per token-half
OOB = 8192           # idx sentinel > NI-1 -> skipped by bounds_check

VERSION = "v4"
_CACHED = {}


_T_PERM = None


def _v1_perm():
    """idx1[t] = nbr[(t//128//24)*128 + t%128, (t//128)%24] as flat index."""
    global _T_PERM
    if _T_PERM is None:
        t = np.arange(NI)
        s, p = t // 128, t % 128
        j, m = (s // Nbr) * 128 + p, s % Nbr
        _T_PERM = j * Nbr + m
    return _T_PERM


def _prep_gidx(nbr16_b):
    idx1 = nbr16_b.reshape(-1)[_v1_perm()]
    return np.tile(idx1.reshape(NI // 16, 16).T, (8, 1))


def _occurrence_tbl(flat, T):
    """tbl[j, t] = flat position of the t-th occurrence of token j."""
    counts = np.bincount(flat, minlength=At)
    order = np.argsort(flat, kind="stable")
    tbl = np.full((At, T), OOB, dtype=np.int32)
    pos = 0
    for j in range(At):
        c = counts[j]
        tbl[j, :c] = order[pos:pos + c]
        pos += c
    return tbl


# ---------------------------------------------------------------- v3 ----
def _prep_v3(nbr16_b, T):
    tbl = _occurrence_tbl(nbr16_b.reshape(-1), T)
    sidx = np.ascontiguousarray(tbl.reshape(2, 128, T).transpose(1, 0, 2))
    return {"gidx": _prep_gidx(nbr16_b), "sidx": sidx}


def _build_nc_v3(T):
    nc = bacc.Bacc("TRN2", target_bir_lowering=False, debug=False)
    emb = nc.dram_tensor("emb", [At, F], mybir.dt.float32, kind="ExternalInput")
    gidx = nc.dram_tensor("gidx", [128, NI // 16], mybir.dt.int16, kind="ExternalInput")
    sidx = nc.dram_tensor("sidx", [128, 2, T], mybir.dt.int32, kind="ExternalInput")
    out = nc.dram_tensor("out", [NI, ROW], mybir.dt.float32, kind="ExternalOutput")

    with tile.TileContext(nc) as tc:
        with tc.tile_pool(name="pool0", bufs=1) as pool0:
            idx_t = pool0.tile([128, NI // 16], mybir.dt.int16)
            nc.sync.dma_start(idx_t[:], gidx[:])
            sidx_t = pool0.tile([128, 2, T], mybir.dt.int32)
            nc.sync.dma_start(sidx_t[:], sidx[:])

            g_t = pool0.tile([128, NI // 128, F], mybir.dt.float32)
            g_scatter = g_t[:].rearrange("p (q m) e -> p q (m e)", q=2)
            for q in range(2):
                nc.gpsimd.dma_gather(
                    g_t[:, q * Nbr:(q + 1) * Nbr, :], emb[:],
                    idx_t[:, q * (HALF // 16):(q + 1) * (HALF // 16)],
                    HALF, HALF, F, single_packet=False,
                )
                src = g_scatter[:, q, :].unsqueeze(1).to_broadcast([128, T, ROW])
                nc.gpsimd.indirect_dma_start(
                    out=out[:],
                    out_offset=bass.IndirectOffsetOnAxis(
                        ap=sidx_t[:, q, :], axis=0),
                    in_=src,
                    in_offset=None,
                    bounds_check=NI - 1,
                    oob_is_err=False,
                )
    nc.compile()
    return nc


# ---------------------------------------------------------------- v4 ----
# Raw bass (no TileContext): identical per-round [128,1]-offset scatters as
# v2, but without Tile's conservative WAW deps between rounds -- the Pool
# engine issues all 2T descriptor-generation ops back-to-back and the 16
# SDMA engines drain continuously.  Sync is manual: idx loads -> gather
# half -> that half's T scatter rounds; final wait on the scatter sem.
def _prep_v4(nbr16_b, T):
    return _prep_v2(nbr16_b, T)


def _build_nc_v4(T):
    nc = bacc.Bacc("TRN2", target_bir_lowering=False, debug=False)
    emb = nc.dram_tensor("emb", [At, F], mybir.dt.float32, kind="ExternalInput")
    gidx = nc.dram_tensor("gidx", [128, NI // 16], mybir.dt.int16, kind="ExternalInput")
    sidx = nc.dram_tensor("sidx", [128, T, 2], mybir.dt.int32, kind="ExternalInput")
    out = nc.dram_tensor("out", [NI, ROW], mybir.dt.float32, kind="ExternalOutput")

    idx_t = nc.alloc_sbuf_tensor("idx_t", [128, NI // 16], mybir.dt.int16)
    sidx_t = nc.alloc_sbuf_tensor("sidx_t", [128, T, 2], mybir.dt.int32)
    g_t = nc.alloc_sbuf_tensor("g_t", [128, NI // 128, F], mybir.dt.float32)

    sem_idx = nc.alloc_semaphore("sem_idx")
    sem_g = nc.alloc_semaphore("sem_g")
    sem_out = nc.alloc_semaphore("sem_out")

    with nc.Block() as blk:

        @blk.sync
        def _(sync):
            sync.dma_start(idx_t[:], gidx[:]).then_inc(sem_idx, 16)
            sync.dma_start(sidx_t[:], sidx[:]).then_inc(sem_idx, 16)

        @blk.gpsimd
        def _(g):
            g.wait_ge(sem_idx, 32)
            g_scatter = g_t[:].rearrange("p (q m) e -> p q (m e)", q=2)
            for q in range(2):
                g.dma_gather(
                    g_t[:, q * Nbr:(q + 1) * Nbr, :], emb[:],
                    idx_t[:, q * (HALF // 16):(q + 1) * (HALF // 16)],
                    HALF, HALF, F, single_packet=False,
                ).then_inc(sem_g, 16)
                g.wait_ge(sem_g, 16 * (q + 1))
                for r in range(T):
                    g.indirect_dma_start(
                        out=out[:],
                        out_offset=bass.IndirectOffsetOnAxis(
                            ap=sidx_t[:, r, q:q + 1], axis=0),
                        in_=g_scatter[:, q, :],
                        in_offset=None,
                        bounds_check=NI - 1,
                        oob_is_err=False,
                    ).then_inc(sem_out, 16)
            g.wait_ge(sem_out, 16 * 2 * T)

    nc.compile()
    return nc


# ---------------------------------------------------------------- v2 ----
def _prep_v2(nbr16_b, T):
    tbl = _occurrence_tbl(nbr16_b.reshape(-1), T)
    sidx = np.empty((128, T, 2), dtype=np.int32)
    for q in range(2):
        sidx[:, :, q] = tbl[q * 128:(q + 1) * 128, :]
    return {"gidx": _prep_gidx(nbr16_b), "sidx": sidx}


def _build_nc_v2(T):
    nc = bacc.Bacc("TRN2", target_bir_lowering=False, debug=False)
    emb = nc.dram_tensor("emb", [At, F], mybir.dt.float32, kind="ExternalInput")
    gidx = nc.dram_tensor("gidx", [128, NI // 16], mybir.dt.int16, kind="ExternalInput")
    sidx = nc.dram_tensor("sidx", [128, T, 2], mybir.dt.int32, kind="ExternalInput")
    out = nc.dram_tensor("out", [NI, ROW], mybir.dt.float32, kind="ExternalOutput")

    with tile.TileContext(nc) as tc:
        with tc.tile_pool(name="pool0", bufs=1) as pool0:
            idx_t = pool0.tile([128, NI // 16], mybir.dt.int16)
            nc.sync.dma_start(idx_t[:], gidx[:])
            sidx_t = pool0.tile([128, T, 2], mybir.dt.int32)
            nc.sync.dma_start(sidx_t[:], sidx[:])

            g_t = pool0.tile([128, NI // 128, F], mybir.dt.float32)
            nc.gpsimd.dma_gather(g_t[:], emb[:], idx_t[:], NI, NI, F,
                                 single_packet=False)

            g_scatter = g_t[:].rearrange("p (q m) e -> p q (m e)", q=2)
            for r in range(T):
                for q in range(2):
                    nc.gpsimd.indirect_dma_start(
                        out=out[:],
                        out_offset=bass.IndirectOffsetOnAxis(
                            ap=sidx_t[:, r, q:q + 1], axis=0),
                        in_=g_scatter[:, q, :],
                        in_offset=None,
                        bounds_check=NI - 1,
                        oob_is_err=False,
                    )
    nc.compile()
    return nc


# ------------------------------------------------------------- driver ----
def _run(nc, in_maps, **kwargs):
    return run_bass_kernel_spmd(nc, in_maps, core_ids=list(range(B)), **kwargs)


def kernel(node_embedding: np.ndarray, nbr_idx: np.ndarray, _collect=None) -> np.ndarray:
    node_embedding = np.ascontiguousarray(node_embedding, dtype=np.float32)
    nbr16 = nbr_idx.astype(np.int16)  # values in [0, 256)

    T = int(max(np.bincount(nbr16[b].reshape(-1), minlength=At).max()
                for b in range(B)))
    key = (VERSION, T)
    builders = {"v2": _build_nc_v2, "v3": _build_nc_v3, "v4": _build_nc_v4}
    preps = {"v2": _prep_v2, "v3": _prep_v3, "v4": _prep_v4}
    if key not in _CACHED:
        _CACHED[key] = builders[VERSION](T)
    nc = _CACHED[key]
    in_maps = [{"emb": node_embedding[b], **preps[VERSION](nbr16[b], T)}
               for b in range(B)]

    res = _run(nc, in_maps)
    if _collect is not None:
        _collect.append(res)
    outs = [res.results[b]["out"].reshape(At, Nbr, Nbr, F) for b in range(B)]
    return np.stack(outs, axis=0)


# revision 11
# speedup vs baseline: 2.1008x; 1.1676x over previous
"""Trainium2 Bass kernel for nn_GetNodeK (gnn_message_passing).

out[b,i,n,m,:] = node_embedding[b, nbr_idx[b, nbr_idx[b,i,n], m], :]

Sharding: data-parallel over B (8 batches -> 8 cores, one batch per core).

Let nbr_flat = nbr_idx[b].reshape(6144) (values < 256) and define the
one-hop table G[j] = concat_m emb[nbr[j,m]] (256 rows x 12 KB = 3.1 MB).
Then out[b, k=(i*24+n)] = G[nbr_flat[k]] -- the 2-hop gather factors into
two index-driven stages that both use the raw nbr values (no chained
index arithmetic anywhere).

v3 (default): stage 1 dma_gather emb->G in SBUF (permuted so scatter-token
j sits at partition j%128, half j//128, 12 KB contiguous), split into two
half gathers (tokens 0-127, 128-255). Stage 2 is ONE indirect_dma_start
per half: offset AP [128, T] (T = max occurrence count; row r of
partition p = r-th output row referencing token q*128+p, OOB-padded) and
a stride-0 broadcast input AP [128, T, 3072] so every descriptor of
partition p reads the same 12 KB row G[q*128+p]. Skipped (OOB) entries
cost only Q7 descriptor-gen time (~1 ns each); real descriptors drain on
all 16 SDMA engines back-to-back instead of the v2 scheme's ~90
serialized 128-descriptor rounds. HBM traffic: 75.5 MB write + 3.1 MB
read per core.

v2 (fallback): T rounds x 2 halves of indirect_dma_start scatter with
[128, 1] offsets -- serialized rounds, ~2.4x slower.
"""
import numpy as np

from concourse import bass, bacc, mybir
import concourse.tile as tile
from concourse.bass_utils import run_bass_kernel_spmd

B, At, Nbr, F = 8, 256, 24, 128
NI = At * Nbr        # 6144 indices per batch
ROW = Nbr * F        # 3072 f32 = 12 KB per stage-2 row
HALF = NI // 2       # 3072 gather indices per token-half
OOB = 8192           # idx sentinel > NI-1 -> skipped by bounds_check

VERSION = "v5"
_CACHED = {}


_T_PERM = None


def _v1_perm():
    """idx1[t] = nbr[(t//128//24)*128 + t%128, (t//128)%24] as flat index."""
    global _T_PERM
    if _T_PERM is None:
        t = np.arange(NI)
        s, p = t // 128, t % 128
        j, m = (s // Nbr) * 128 + p, s % Nbr
        _T_PERM = j * Nbr + m
    return _T_PERM


def _prep_gidx(nbr16_b):
    idx1 = nbr16_b.reshape(-1)[_v1_perm()]
    return np.tile(idx1.reshape(NI // 16, 16).T, (8, 1))


def _occurrence_tbl(flat, T):
    """tbl[j, t] = flat position of the t-th occurrence of token j."""
    counts = np.bincount(flat, minlength=At)
    order = np.argsort(flat, kind="stable")
    tbl = np.full((At, T), OOB, dtype=np.int32)
    pos = 0
    for j in range(At):
        c = counts[j]
        tbl[j, :c] = order[pos:pos + c]
        pos += c
    return tbl


# ---------------------------------------------------------------- v3 ----
def _prep_v3(nbr16_b, T):
    tbl = _occurrence_tbl(nbr16_b.reshape(-1), T)
    sidx = np.ascontiguousarray(tbl.reshape(2, 128, T).transpose(1, 0, 2))
    return {"gidx": _prep_gidx(nbr16_b), "sidx": sidx}


def _build_nc_v3(T):
    nc = bacc.Bacc("TRN2", target_bir_lowering=False, debug=False)
    emb = nc.dram_tensor("emb", [At, F], mybir.dt.float32, kind="ExternalInput")
    gidx = nc.dram_tensor("gidx", [128, NI // 16], mybir.dt.int16, kind="ExternalInput")
    sidx = nc.dram_tensor("sidx", [128, 2, T], mybir.dt.int32, kind="ExternalInput")
    out = nc.dram_tensor("out", [NI, ROW], mybir.dt.float32, kind="ExternalOutput")

    with tile.TileContext(nc) as tc:
        with tc.tile_pool(name="pool0", bufs=1) as pool0:
            idx_t = pool0.tile([128, NI // 16], mybir.dt.int16)
            nc.sync.dma_start(idx_t[:], gidx[:])
            sidx_t = pool0.tile([128, 2, T], mybir.dt.int32)
            nc.sync.dma_start(sidx_t[:], sidx[:])

            g_t = pool0.tile([128, NI // 128, F], mybir.dt.float32)
            g_scatter = g_t[:].rearrange("p (q m) e -> p q (m e)", q=2)
            for q in range(2):
                nc.gpsimd.dma_gather(
                    g_t[:, q * Nbr:(q + 1) * Nbr, :], emb[:],
                    idx_t[:, q * (HALF // 16):(q + 1) * (HALF // 16)],
                    HALF, HALF, F, single_packet=False,
                )
                src = g_scatter[:, q, :].unsqueeze(1).to_broadcast([128, T, ROW])
                nc.gpsimd.indirect_dma_start(
                    out=out[:],
                    out_offset=bass.IndirectOffsetOnAxis(
                        ap=sidx_t[:, q, :], axis=0),
                    in_=src,
                    in_offset=None,
                    bounds_check=NI - 1,
                    oob_is_err=False,
                )
    nc.compile()
    return nc


# ---------------------------------------------------------------- v4 ----
# Raw bass (no TileContext): identical per-round [128,1]-offset scatters as
# v2, but without Tile's conservative WAW deps between rounds -- the Pool
# engine issues all 2T descriptor-generation ops back-to-back and the 16
# SDMA engines drain continuously.  Sync is manual: idx loads -> gather
# half -> that half's T scatter rounds; final wait on the scatter sem.
def _prep_v4(nbr16_b, T):
    return _prep_v2(nbr16_b, T)


def _build_nc_v4(T):
    nc = bacc.Bacc("TRN2", target_bir_lowering=False, debug=False)
    emb = nc.dram_tensor("emb", [At, F], mybir.dt.float32, kind="ExternalInput")
    gidx = nc.dram_tensor("gidx", [128, NI // 16], mybir.dt.int16, kind="ExternalInput")
    sidx = nc.dram_tensor("sidx", [128, T, 2], mybir.dt.int32, kind="ExternalInput")
    out = nc.dram_tensor("out", [NI, ROW], mybir.dt.float32, kind="ExternalOutput")

    idx_t = nc.alloc_sbuf_tensor("idx_t", [128, NI // 16], mybir.dt.int16)
    sidx_t = nc.alloc_sbuf_tensor("sidx_t", [128, T, 2], mybir.dt.int32)
    g_t = nc.alloc_sbuf_tensor("g_t", [128, NI // 128, F], mybir.dt.float32)

    sem_idx = nc.alloc_semaphore("sem_idx")
    sem_g = nc.alloc_semaphore("sem_g")
    sem_out = nc.alloc_semaphore("sem_out")

    with nc.Block() as blk:

        @blk.sync
        def _(sync):
            sync.dma_start(idx_t[:], gidx[:]).then_inc(sem_idx, 16)
            sync.dma_start(sidx_t[:], sidx[:]).then_inc(sem_idx, 16)

        @blk.gpsimd
        def _(g):
            g.wait_ge(sem_idx, 32)
            g_scatter = g_t[:].rearrange("p (q m) e -> p q (m e)", q=2)
            for q in range(2):
                g.dma_gather(
                    g_t[:, q * Nbr:(q + 1) * Nbr, :], emb[:],
                    idx_t[:, q * (HALF // 16):(q + 1) * (HALF // 16)],
                    HALF, HALF, F, single_packet=False,
                ).then_inc(sem_g, 16)
                g.wait_ge(sem_g, 16 * (q + 1))
                for r in range(T):
                    g.indirect_dma_start(
                        out=out[:],
                        out_offset=bass.IndirectOffsetOnAxis(
                            ap=sidx_t[:, r, q:q + 1], axis=0),
                        in_=g_scatter[:, q, :],
                        in_offset=None,
                        bounds_check=NI - 1,
                        oob_is_err=False,
                    ).then_inc(sem_out, 16)
            g.wait_ge(sem_out, 16 * 2 * T)

    nc.compile()
    return nc


# ---------------------------------------------------------------- v5 ----
# v4 + stage-1 gather moved off the GpSimd/DMA path entirely: G is built by
# TensorE permutation matmuls.  Host uploads exact fp16 one-hot matrices
# PT[(q*24+m)*2+h][i, j] = (nbr[q*128+j, m] == h*128+i); per (q,m) tile
# G[j, :] = PT_lo.T @ emb_lo + PT_hi.T @ emb_hi accumulates in PSUM (f32,
# exactly one nonzero term -> result is just emb rounded to fp16, rel err
# ~2^-11 << 2e-2 gate).  DVE copies PSUM->SBUF.  The Pool engine runs ONLY
# the 2T indirect-scatter rounds, and the 16 SDMA engines carry nothing but
# the 75.5 MB output write.
NT = 2 * Nbr         # 48 (q,m) tiles
NG = NT // 4         # 12 groups of 4 tiles (one PSUM bank each)


def _prep_v5(nbr16_b, T):
    nbr_r = nbr16_b.reshape(2, 128, Nbr).astype(np.int64)  # [q, j, m]
    pt = np.zeros((128, 2, Nbr, 2, 128), dtype=np.float16)  # [i, q, m, h, j]
    q_ix, j_ix, m_ix = np.meshgrid(np.arange(2), np.arange(128),
                                   np.arange(Nbr), indexing="ij")
    vals = nbr_r[q_ix, j_ix, m_ix]
    pt[vals % 128, q_ix, m_ix, vals // 128, j_ix] = np.float16(1.0)
    ptd = np.ascontiguousarray(pt.reshape(128, NT * 2, 128))

    tbl = _occurrence_tbl(nbr16_b.reshape(-1), T)
    sidx = np.empty((128, T, 2), dtype=np.int32)
    for q in range(2):
        sidx[:, :, q] = tbl[q * 128:(q + 1) * 128, :]
    return {"ptd": ptd, "sidx": sidx}


def _build_nc_v5(T):
    nc = bacc.Bacc("TRN2", target_bir_lowering=False, debug=False)
    emb16d = nc.dram_tensor("emb16", [128, 2, F], mybir.dt.float16, kind="ExternalInput")
    ptd = nc.dram_tensor("ptd", [128, NT * 2, 128], mybir.dt.float16, kind="ExternalInput")
    sidxd = nc.dram_tensor("sidx", [128, T, 2], mybir.dt.int32, kind="ExternalInput")
    out = nc.dram_tensor("out", [NI, ROW], mybir.dt.float32, kind="ExternalOutput")

    emb_t = nc.alloc_sbuf_tensor("emb_t", [128, 2, F], mybir.dt.float16)
    pt_t = nc.alloc_sbuf_tensor("pt_t", [128, NT * 2, 128], mybir.dt.float16)
    sidx_t = nc.alloc_sbuf_tensor("sidx_t", [128, T, 2], mybir.dt.int32)
    g_t = nc.alloc_sbuf_tensor("g_t", [128, NI // 128, F], mybir.dt.float32)
    ps = nc.alloc_psum_tensor("ps", [128, 8, 128], mybir.dt.float32)

    sem_in = nc.alloc_semaphore("sem_in")
    sem_in2 = nc.alloc_semaphore("sem_in2")
    sem_sidx = nc.alloc_semaphore("sem_sidx")
    sem_pe = nc.alloc_semaphore("sem_pe")
    sem_dve = nc.alloc_semaphore("sem_dve")
    sem_out = nc.alloc_semaphore("sem_out")

    with nc.Block() as blk:

        @blk.sync
        def _(sync):
            sync.dma_start(emb_t[:], emb16d[:]).then_inc(sem_in, 16)
            # pt halves separately so PE can start on half 0 sooner
            sync.dma_start(pt_t[:, :NT, :], ptd[:, :NT, :]).then_inc(sem_in, 16)
            sync.dma_start(pt_t[:, NT:, :], ptd[:, NT:, :]).then_inc(sem_in2, 16)
            sync.dma_start(sidx_t[:], sidxd[:]).then_inc(sem_sidx, 16)

        @blk.tensor
        def _(te):
            te.wait_ge(sem_in, 32)  # emb + pt half 0
            for g in range(NG):
                if g == NG // 2:
                    te.wait_ge(sem_in2, 16)  # pt half 1
                if g >= 2:
                    te.wait_ge(sem_dve, g - 1)  # bank g%2 reusable
                bank = g % 2
                for k in range(4):
                    s = 4 * g + k
                    te.matmul(out=ps[:, 4 * bank + k, :],
                              lhsT=pt_t[:, 2 * s, :], rhs=emb_t[:, 0, :],
                              start=True, stop=False)
                    mm = te.matmul(out=ps[:, 4 * bank + k, :],
                                   lhsT=pt_t[:, 2 * s + 1, :], rhs=emb_t[:, 1, :],
                                   start=False, stop=True)
                    if k == 3:
                        mm.then_inc(sem_pe, 1)

        @blk.vector
        def _(ve):
            for g in range(NG):
                ve.wait_ge(sem_pe, g + 1)
                bank = g % 2
                ve.tensor_copy(
                    out=g_t[:, 4 * g:4 * g + 4, :],
                    in_=ps[:, 4 * bank:4 * bank + 4, :],
                ).then_inc(sem_dve, 1)

        @blk.gpsimd
        def _(g):
            g_scatter = g_t[:].rearrange("p (q m) e -> p q (m e)", q=2)
            g.wait_ge(sem_sidx, 16)
            for q in range(2):
                g.wait_ge(sem_dve, (NG // 2) * (q + 1))
                for r in range(T):
                    g.indirect_dma_start(
                        out=out[:],
                        out_offset=bass.IndirectOffsetOnAxis(
                            ap=sidx_t[:, r, q:q + 1], axis=0),
                        in_=g_scatter[:, q, :],
                        in_offset=None,
                        bounds_check=NI - 1,
                        oob_is_err=False,
                    ).then_inc(sem_out, 16)
            g.wait_ge(sem_out, 16 * 2 * T)

    nc.compile()
    return nc


# ---------------------------------------------------------------- v2 ----
def _prep_v2(nbr16_b, T):
    tbl = _occurrence_tbl(nbr16_b.reshape(-1), T)
    sidx = np.empty((128, T, 2), dtype=np.int32)
    for q in range(2):
        sidx[:, :, q] = tbl[q * 128:(q + 1) * 128, :]
    return {"gidx": _prep_gidx(nbr16_b), "sidx": sidx}


def _build_nc_v2(T):
    nc = bacc.Bacc("TRN2", target_bir_lowering=False, debug=False)
    emb = nc.dram_tensor("emb", [At, F], mybir.dt.float32, kind="ExternalInput")
    gidx = nc.dram_tensor("gidx", [128, NI // 16], mybir.dt.int16, kind="ExternalInput")
    sidx = nc.dram_tensor("sidx", [128, T, 2], mybir.dt.int32, kind="ExternalInput")
    out = nc.dram_tensor("out", [NI, ROW], mybir.dt.float32, kind="ExternalOutput")

    with tile.TileContext(nc) as tc:
        with tc.tile_pool(name="pool0", bufs=1) as pool0:
            idx_t = pool0.tile([128, NI // 16], mybir.dt.int16)
            nc.sync.dma_start(idx_t[:], gidx[:])
            sidx_t = pool0.tile([128, T, 2], mybir.dt.int32)
            nc.sync.dma_start(sidx_t[:], sidx[:])

            g_t = pool0.tile([128, NI // 128, F], mybir.dt.float32)
            nc.gpsimd.dma_gather(g_t[:], emb[:], idx_t[:], NI, NI, F,
                                 single_packet=False)

            g_scatter = g_t[:].rearrange("p (q m) e -> p q (m e)", q=2)
            for r in range(T):
                for q in range(2):
                    nc.gpsimd.indirect_dma_start(
                        out=out[:],
                        out_offset=bass.IndirectOffsetOnAxis(
                            ap=sidx_t[:, r, q:q + 1], axis=0),
                        in_=g_scatter[:, q, :],
                        in_offset=None,
                        bounds_check=NI - 1,
                        oob_is_err=False,
                    )
    nc.compile()
    return nc


# ------------------------------------------------------------- driver ----
def _run(nc, in_maps, **kwargs):
    return run_bass_kernel_spmd(nc, in_maps, core_ids=list(range(B)), **kwargs)


def kernel(node_embedding: np.ndarray, nbr_idx: np.ndarray, _collect=None) -> np.ndarray:
    node_embedding = np.ascontiguousarray(node_embedding, dtype=np.float32)
    nbr16 = nbr_idx.astype(np.int16)  # values in [0, 256)

    T = int(max(np.bincount(nbr16[b].reshape(-1), minlength=At).max()
                for b in range(B)))
    key = (VERSION, T)
    builders = {"v2": _build_nc_v2, "v3": _build_nc_v3, "v4": _build_nc_v4,
                "v5": _build_nc_v5}
    preps = {"v2": _prep_v2, "v3": _prep_v3, "v4": _prep_v4, "v5": _prep_v5}
    if key not in _CACHED:
        _CACHED[key] = builders[VERSION](T)
    nc = _CACHED[key]
    if VERSION == "v5":
        in_maps = [{"emb16": np.ascontiguousarray(
                        node_embedding[b].reshape(2, 128, F)
                        .transpose(1, 0, 2)).astype(np.float16),
                    **_prep_v5(nbr16[b], T)}
                   for b in range(B)]
    else:
        in_maps = [{"emb": node_embedding[b], **preps[VERSION](nbr16[b], T)}
                   for b in range(B)]

    res = _run(nc, in_maps)
    if _collect is not None:
        _collect.append(res)
    outs = [res.results[b]["out"].reshape(At, Nbr, Nbr, F) for b in range(B)]
    return np.stack(outs, axis=0)
